# revision 18
# baseline (speedup 1.0000x reference)
"""CQAttention Bass kernel for TRN2, 8 NeuronCores, batch-parallel. v4.

Shapes: context [16,128,2048] f32, query [16,128,512] f32, w [384] f32
-> out [16,512,2048] f32.  2 batches per core, batch-sharded (no comms).

Per batch (D=128, C=2048, Q=512):
  sT[q,c] = sum_d qryW2[d,q] ctx[d,c]     (qryW2 = qry*wcq + wc folds bias_c;
                                           bias_q cancels in softmax over c)
  ET      = exp(sT - KBIAS)               [q-part, c] bf16 (jq3: fp8 + accum)
  ET8     = fp8(ET)                       (Pool casts; DoubleRow a/b2 rhs)
  E       = ET^T                          [c-part, q] bf16 (DMA transpose for
                                          jq0/1, PE transpose for jq2/3)
  t[q,d]  = sum_c E[c,q] ctxC[c,d]        bf16; ctxC col 128 = ones, so col
                                          128 of t = S[q] (softmax denom)
  t28     = t * recip^2 * ST (fp8), qryR8 = qryT * recip * SQ (fp8)
  aT[d,c] = (sum_q qryR8[q,d] ET8[q,c]) / SQ        (fp8 DoubleRow)
  b2T[d,c]= sum_q t28[q,d] ET8[q,c]                 (fp8 DoubleRow; /ST on
                                                     host)
  out     = [ctxT; aT; ctxT*aT; ctxT*b2T]  stored bf16, host upcasts to f32

Key structure:
  - inputs host-cast to bf16 (half load traffic, no device casts); outputs
    bf16 (sections 2-4 carry ~2% of the output norm; total rel err ~2e-3
    against the 2e-2 gate)
  - section 1 = DRAM->DRAM DMA, fires immediately
  - GPSIMD cannot read PSUM, so Pool does only SBUF->SBUF work (ET8 casts,
    qryR8, qryW2, sec3); all PSUM unloading is DVE/Act
  - qryT and E(jq0/1) transposes ride the idle DMA engines (dma transpose);
  - jq3 keeps an fp8 exp output + exact-accum S so the tail chain skips both
    the cast and the E-copy wait for its softmax scales
"""

import contextlib as _cl

import numpy as np
import ml_dtypes

import concourse.bass as bass
import concourse.mybir as mybir
import concourse.tile as tile
from concourse.bass import ts, ds
from concourse.bass_utils import run_bass_kernel_spmd
from concourse.masks import make_identity

B, D, C, Q = 16, 128, 2048, 512
NCORES = 8
BPC = B // NCORES
NCT = C // 128
NQT = Q // 128
NCH = C // 512
F32 = mybir.dt.float32
BF16 = mybir.dt.bfloat16
FP8 = mybir.dt.float8e4
AF = mybir.ActivationFunctionType
MULT = mybir.AluOpType.mult
ADD = mybir.AluOpType.add
DR = mybir.MatmulPerfMode.DoubleRow

KBIAS = 1.5
SQ = 128.0
ST = 4096.0

_SPLIT_TYPES = (
    "InstMatmult", "InstLdweights", "InstActivation", "InstTensorScalar",
    "InstTensorScalarPtr", "InstTensorScalarAffineSelect", "InstTensorTensor",
    "InstTensorCopy", "InstReciprocal", "InstMemset", "InstCopyPredicated",
    "InstBNStats", "InstStreamTranspose", "InstTensorReduce", "InstIota",
    "InstDMACopy", "InstDMA", "InstDMAGather", "InstDMAGatherAnt",
    "InstDmaTransposeAnt", "InstDrain",
)


def _split_multi_waits(nc, max_embedded=1):
    """walrus allows very few embedded sync-waits per compute instruction.
    Move a Matmult's extra waits onto its adjacent Ldweights (engine-path,
    zero cost); hoist the rest into seq EventSemaphore nops."""
    n = 0
    for fn in nc.m.functions:
        for blk in fn.blocks:
            il = blk.instructions
            i = 0
            while i < len(il):
                inst = il[i]
                si = inst.sync_info
                if (si is None or not si.on_wait
                        or len(si.on_wait) <= max_embedded
                        or type(inst).__name__ not in _SPLIT_TYPES):
                    i += 1
                    continue
                waits = list(si.on_wait)
                extra, keep = waits[:-max_embedded], waits[-max_embedded:]
                if (type(inst).__name__ == "InstMatmult" and i > 0
                        and type(il[i - 1]).__name__ == "InstLdweights"
                        and il[i - 1].engine == inst.engine):
                    ld = il[i - 1]
                    lsi = ld.sync_info
                    lw = list(lsi.on_wait) if (lsi and lsi.on_wait) else []
                    room = max_embedded - len(lw)
                    if room > 0:
                        take, extra = extra[:room], extra[room:]
                        lu = list(lsi.on_update) if (lsi and lsi.on_update) \
                            else []
                        ld.sync_info = mybir.SyncInfo(on_wait=lw + take,
                                                      on_update=lu)
                for k, w in enumerate(extra):
                    nop = mybir.InstEventSemaphore(
                        name=f"{inst.name}-w{k}", engine=inst.engine,
                        ins=[], outs=[])
                    nop.sync_info = mybir.SyncInfo(on_wait=[w], on_update=[])
                    il.insert(i, nop)
                    i += 1
                    n += 1
                inst.sync_info = mybir.SyncInfo(on_wait=keep,
                                                on_update=si.on_update)
                i += 1
    return n


def build_kernel():
    nc = bass.Bass("TRN2", target_bir_lowering=False, debug=False,
                   num_devices=NCORES)
    ctx_ext = nc.dram_tensor("context", [BPC, D, C], BF16,
                             kind="ExternalInput").ap()
    qry_ext = nc.dram_tensor("query", [BPC, D, Q], BF16,
                             kind="ExternalInput").ap()
    w_ext = nc.dram_tensor("w", [3 * D], F32, kind="ExternalInput").ap()
    out_ext = nc.dram_tensor("out", [BPC, 4 * D, C], BF16,
                             kind="ExternalOutput").ap()

    st = {}

    with tile.TileContext(nc) as tc:
        with _cl.ExitStack() as ex:
            singles = ex.enter_context(tc.tile_pool(name="singles", bufs=1))
            bb = ex.enter_context(tc.tile_pool(name="bb", bufs=2))
            ps_s = ex.enter_context(
                tc.tile_pool(name="ps_s", bufs=2, space="PSUM"))
            ps_et = ex.enter_context(
                tc.tile_pool(name="ps_et", bufs=2, space="PSUM"))
            ps_misc = ex.enter_context(
                tc.tile_pool(name="ps_misc", bufs=2, space="PSUM"))

            # ---- prologue ----
            ident_bf = singles.tile([128, 128], BF16)
            make_identity(nc, ident_bf)
            ident_f8 = singles.tile([128, 128], FP8)
            make_identity(nc, ident_f8)
            p_w = ps_misc.tile([128, 8, 128], BF16, tag="misc")
            for k in range(12):
                nc.tensor.transpose(p_w[:, k % 8, :], ident_bf, ident_bf)
            ident_chk = singles.tile([128, 128], BF16)
            nc.vector.tensor_copy(ident_chk, p_w[:, 0, :])
            wcols = singles.tile([128, 2], F32)
            with tc.high_priority():
                nc.sync.dma_start(
                    out=wcols,
                    in_=w_ext[ds(D, 2 * D)].rearrange("(o p) -> p o", o=2))
            wc_col = wcols[:, 0:1]
            wcq_col = wcols[:, 1:2]
            nbias = singles.tile([128, 1], F32)
            nc.vector.memset(nbias, -KBIAS)

            # ---- loads ----
            for b in range(BPC):
                s = st[b] = {}
                s["cb"] = bb.tile([128, C], BF16, tag="ctxbf", name=f"cb{b}")
                s["qb"] = bb.tile([128, Q], BF16, tag="qrybf", name=f"qb{b}")
            with tc.high_priority():
                nc.sync.dma_start(out=st[0]["qb"], in_=qry_ext[0])
            for b in range(BPC):
                prio = tc.high_priority() if b == 0 else _cl.nullcontext()
                with prio:
                    if b:
                        nc.sync.dma_start(out=st[b]["qb"], in_=qry_ext[b])
                    if b == 0:
                        for q4 in range(2):
                            nc.sync.dma_start(
                                out=st[0]["cb"][:, ds(q4 * 512, 512)],
                                in_=ctx_ext[0][:, ds(q4 * 512, 512)])
                        nc.sync.dma_start(
                            out=st[0]["cb"][:, ds(1024, 1024)],
                            in_=ctx_ext[0][:, ds(1024, 1024)])
                    else:
                        for h in range(2):
                            nc.sync.dma_start(
                                out=st[b]["cb"][:, ds(h * 1024, 1024)],
                                in_=ctx_ext[b][:, ds(h * 1024, 1024)])
            for b in range(BPC):
                nc.sync.dma_start(out=out_ext[b, 0:D, :], in_=ctx_ext[b])

            def f_qryW2(b):
                s = st[b]
                s["qryW2"] = bb.tile([128, Q], BF16, tag="qryW2",
                                     name=f"qryW2_{b}")
                nc.vector.tensor_scalar(
                    out=s["qryW2"], in0=s["qb"], scalar1=wcq_col,
                    scalar2=wc_col, op0=MULT, op1=ADD)

            def f_qryT(b):
                # qryT via DMA transpose: no PE/PSUM involvement
                s = st[b]
                s["qryT"] = bb.tile([128, NQT, 128], BF16, tag="qryT",
                                    name=f"qryT_{b}")
                nc.sync.dma_start_transpose(s["qryT"], s["qb"])

            def f_ctxC(b, g):
                # PE transpose (bf16) + DVE 2x copy; col 128 = ones so the
                # t-matmul's 129th column accumulates S[q]
                s = st[b]
                if "ctxC" not in s:
                    s["ctxC"] = bb.tile([128, NCT, 129], BF16, tag="ctxC",
                                        name=f"ctxC_{b}")
                    nc.vector.memset(s["ctxC"][:, :, 128:129], 1.0)
                p_ct = ps_misc.tile([128, 8, 128], BF16, tag="misc")
                for jj in range(8):
                    nc.tensor.transpose(
                        p_ct[:, jj, :], s["cb"][:, ts(g * 8 + jj, 128)],
                        ident_bf)
                nc.vector.tensor_copy(s["ctxC"][:, ds(g * 8, 8), 0:128],
                                      p_ct)

            def f_s(b, jq):
                # jq<3 -> bf16 ET (cast to fp8 later, transposed for E);
                # jq3  -> fp8 ET directly + exact-accum S
                s = st[b]
                if "ET" not in s:
                    s["ET"] = bb.tile([128, 3, C], BF16, tag="ET",
                                      name=f"ET_{b}")
                    s["ET8"] = bb.tile([128, NQT, C], FP8, tag="ET8",
                                       name=f"ET8_{b}")
                    s["Spart"] = bb.tile([128, 2, 2], F32, tag="Spart",
                                         name=f"Spart{b}")
                for h in range(2):
                    p_sT = ps_s.tile([128, 1024], F32, tag="ps_s")
                    for k in range(2):
                        nc.tensor.matmul(
                            p_sT[:, ts(k, 512)],
                            lhsT=s["qryW2"][:, ts(jq, 128)],
                            rhs=s["cb"][:, ds(h * 1024 + k * 512, 512)],
                            start=True, stop=True)
                    if jq == 3:
                        acc = (s["Spart"][:, 1, h].unsqueeze(1)
                               if b == 1 else None)
                        nc.scalar.activation(
                            s["ET8"][:, 3, ds(h * 1024, 1024)], p_sT,
                            AF.Exp, bias=nbias, accum_out=acc)
                    elif jq == 2 and b == 1:
                        nc.scalar.activation(
                            s["ET"][:, 2, ds(h * 1024, 1024)], p_sT,
                            AF.Exp, bias=nbias,
                            accum_out=s["Spart"][:, 0, h].unsqueeze(1))
                    else:
                        nc.scalar.activation(
                            s["ET"][:, jq, ds(h * 1024, 1024)], p_sT,
                            AF.Exp, bias=nbias)

            def f_cast8(b, jq):
                # bf16 -> fp8 cast of one ET row block (SBUF->SBUF)
                s = st[b]
                if b == 1 and jq == 2:
                    # tail critical: halves on Pool + DVE in parallel
                    nc.gpsimd.tensor_copy(s["ET8"][:, 2, 0:1024],
                                          s["ET"][:, 2, 0:1024])
                    nc.vector.tensor_copy(s["ET8"][:, 2, 1024:2048],
                                          s["ET"][:, 2, 1024:2048])
                    return
                nc.gpsimd.tensor_copy(s["ET8"][:, jq, :], s["ET"][:, jq, :])

            def f_etr_dma(b, jq):
                # E(jq) via DMA transpose (bf16, SBUF->SBUF, no copies);
                # high priority so stores never precede it on the SP queue
                s = st[b]
                if "E" not in s:
                    s["E"] = bb.tile([128, NCT, Q], BF16, tag="E",
                                     name=f"E_{b}")
                nc.sync.dma_start_transpose(
                    s["E"][:, :, ts(jq, 128)], s["ET"][:, jq, :])

            def f_etr_pe(b, jq):
                # E(jq) via PE transposes; jq3's source is fp8 (stride-2
                # PSUM layout per walrus), copies destride + upcast to bf16
                s = st[b]
                if "E" not in s:
                    s["E"] = bb.tile([128, NCT, Q], BF16, tag="E",
                                     name=f"E_{b}")
                fp8src = jq == 3
                for g in range(2):
                    if fp8src:
                        p_et = ps_et.tile([128, 8, 128, 2], FP8, tag="et")
                        for jj in range(8):
                            nc.tensor.transpose(
                                p_et[:, jj, :, 0],
                                s["ET8"][:, 3, ts(g * 8 + jj, 128)],
                                ident_f8)
                        src = p_et[:, :, :, 0]
                    else:
                        p_et = ps_et.tile([128, 8, 128], BF16, tag="et")
                        for jj in range(8):
                            nc.tensor.transpose(
                                p_et[:, jj, :],
                                s["ET"][:, jq, ts(g * 8 + jj, 128)],
                                ident_bf)
                        src = p_et
                    ceng = nc.scalar if b == 1 else nc.vector
                    if ceng is nc.scalar:
                        nc.scalar.copy(s["E"][:, ds(g * 8, 8), ts(jq, 128)],
                                       src)
                    else:
                        nc.vector.tensor_copy(
                            s["E"][:, ds(g * 8, 8), ts(jq, 128)], src)

            def f_scales(b, jq, Scol):
                s = st[b]
                jj = ds(jq, 1)
                ex2 = tc.high_priority()
                ex2.__enter__()
                recip = bb.tile([128, 1], F32, tag="recip",
                                name=f"recip{b}_{jq}")
                nc.vector.reciprocal(recip, Scol)
                nc.vector.tensor_scalar_mul(s["recipQ"][:, jj], recip, SQ)
                recip2 = bb.tile([128, 1], F32, tag="recip2",
                                 name=f"recip2{b}_{jq}")
                nc.vector.tensor_tensor(out=recip2, in0=recip, in1=recip,
                                        op=MULT)
                nc.vector.tensor_scalar_mul(s["recipT2"][:, jj], recip2, ST)
                nc.gpsimd.tensor_scalar_mul(
                    s["qryR8"][:, jq, :], s["qryT"][:, jq, :],
                    s["recipQ"][:, jq:jq + 1])
                ex2.__exit__(None, None, None)

            def _mkscales(s, b):
                if "recipQ" not in s:
                    s["recipQ"] = bb.tile([128, NQT], F32, tag="recipQ",
                                          name=f"recipQ{b}")
                    s["recipT2"] = bb.tile([128, NQT], F32, tag="recipT2",
                                           name=f"recipT2{b}")
                    s["qryR8"] = bb.tile([128, NQT, 128], FP8, tag="qryR8",
                                         name=f"qryR8_{b}")
                    s["t28"] = bb.tile([128, NQT, 128], FP8, tag="t28",
                                       name=f"t28_{b}")

            def f_recip23(b):
                s = st[b]
                _mkscales(s, b)
                if b == 0:
                    return  # b0 scales come from the t-chain ones-columns
                Ssum = bb.tile([128, 2], F32, tag="Ssum", name=f"Ssum{b}")
                nc.vector.tensor_tensor(out=Ssum, in0=s["Spart"][:, :, 0],
                                        in1=s["Spart"][:, :, 1], op=ADD)
                f_scales(b, 2, Ssum[:, 0:1])
                f_scales(b, 3, Ssum[:, 1:2])

            def f_t(b, jq):
                # all-bf16 t chain; col 128 of the output is S[q] (jq<3)
                s = st[b]
                _mkscales(s, b)
                p_t = ps_misc.tile([128, 129], F32, tag="misc",
                                   name=f"p_t{b}_{jq}")
                for jc in range(NCT):
                    nc.tensor.matmul(
                        p_t, lhsT=s["E"][:, jc, ts(jq, 128)],
                        rhs=s["ctxC"][:, jc, :],
                        start=(jc == 0), stop=(jc == NCT - 1))
                if jq < 2 or (b == 0 and jq < 4):
                    f_scales(b, jq, p_t[:, 128:129])
                nc.vector.tensor_scalar_mul(
                    s["t28"][:, jq, :], p_t[:, 0:128],
                    s["recipT2"][:, jq:jq + 1])

            def f_a(b, jch):
                s = st[b]
                if "a_bf" not in s:
                    s["a_bf"] = bb.tile([128, C], BF16, tag="a_bf",
                                        name=f"a_bf{b}")
                    s["sec3"] = bb.tile([128, C], BF16, tag="sec3",
                                        name=f"sec3_{b}")
                sl = ts(jch, 512)
                pool = ps_s if b == 1 else ps_et
                p_a = pool.tile([128, 512], F32,
                                tag="ps_s" if b == 1 else "et")
                for u in range(2):
                    nc.tensor.matmul(
                        p_a, lhsT=s["qryR8"][:, ds(2 * u, 2), :],
                        rhs=s["ET8"][:, ds(2 * u, 2), sl],
                        start=(u == 0), stop=(u == 1), perf_mode=DR)
                aeng = nc.scalar if b == 1 else nc.vector
                if aeng is nc.scalar:
                    nc.scalar.mul(s["a_bf"][:, sl], p_a, 1.0 / SQ)
                else:
                    nc.vector.tensor_scalar_mul(s["a_bf"][:, sl], p_a,
                                                1.0 / SQ)
                seng = nc.gpsimd if b == 0 else nc.vector
                seng.tensor_tensor(
                    out=s["sec3"][:, sl], in0=s["cb"][:, sl],
                    in1=s["a_bf"][:, sl], op=MULT)

            def f_b(b, jch):
                # sec4 = ctx * (b2*ST) straight from PSUM (DVE); /ST on host
                s = st[b]
                if "sec4" not in s:
                    s["sec4"] = bb.tile([128, C], BF16, tag="sec4",
                                        name=f"sec4_{b}")
                sl = ts(jch, 512)
                pool = ps_s if b == 1 else ps_misc
                p_b = pool.tile([128, 512], F32,
                                tag="ps_s" if b == 1 else "misc")
                for u in range(2):
                    nc.tensor.matmul(
                        p_b, lhsT=s["t28"][:, ds(2 * u, 2), :],
                        rhs=s["ET8"][:, ds(2 * u, 2), sl],
                        start=(u == 0), stop=(u == 1), perf_mode=DR)
                nc.vector.tensor_tensor(
                    out=s["sec4"][:, sl], in0=s["cb"][:, sl],
                    in1=p_b, op=MULT)

            def f_store(b, sec, h):
                s = st[b]
                key = {2: "a_bf", 3: "sec3", 4: "sec4"}[sec]
                src = s[key]
                hs = ds(h * 1024, 1024)
                eng = nc.scalar if sec == 4 else nc.sync
                eng.dma_start(out=out_ext[b, ds((sec - 1) * D, D), hs],
                              in_=src[:, hs])

            # ---- software-pipelined emission ----
            f_qryW2(0)
            f_qryW2(1)
            f_qryT(0)
            f_s(0, 0)
            f_s(0, 1)
            f_cast8(0, 0)
            f_ctxC(0, 0)
            f_etr_dma(0, 0)
            f_s(0, 2)
            f_ctxC(0, 1)
            f_cast8(0, 1)
            f_qryT(1)
            f_etr_dma(0, 1)
            f_s(0, 3)
            f_cast8(0, 2)
            f_ctxC(1, 0)
            f_etr_dma(0, 2)
            f_recip23(0)
            f_s(1, 0)
            f_etr_pe(0, 3)
            f_ctxC(1, 1)
            f_t(0, 0)
            f_s(1, 1)
            f_cast8(1, 0)
            f_t(0, 1)
            f_etr_dma(1, 0)
            f_s(1, 2)
            f_cast8(1, 1)
            f_t(0, 2)
            f_etr_dma(1, 1)
            f_t(0, 3)
            f_a(0, 0)
            f_a(0, 1)
            f_s(1, 3)
            f_a(0, 2)
            f_a(0, 3)
            f_recip23(1)
            for jch in range(NCH):
                f_b(0, jch)
            f_cast8(1, 2)
            f_etr_dma(1, 2)
            f_store(0, 2, 0)
            f_store(0, 3, 0)
            f_store(0, 2, 1)
            f_store(0, 3, 1)
            f_etr_pe(1, 3)
            f_store(0, 4, 0)
            f_store(0, 4, 1)
            f_t(1, 0)
            f_t(1, 1)
            for jch in range(NCH):
                f_a(1, jch)
            f_t(1, 2)
            f_store(1, 2, 0)
            f_store(1, 3, 0)
            f_t(1, 3)
            for jch in range(NCH):
                f_b(1, jch)
            f_store(1, 2, 1)
            f_store(1, 3, 1)
            for qh in range(4):
                s = st[1]
                nc.scalar.dma_start(
                    out=out_ext[1, ds(3 * D, D), ts(qh, 512)],
                    in_=s["sec4"][:, ts(qh, 512)])
    _split_multi_waits(nc)
    return nc


_NC = None


def kernel(context: np.ndarray, query: np.ndarray, w: np.ndarray,
           **extra) -> np.ndarray:
    global _NC
    if _NC is None:
        _NC = build_kernel()
    context = np.ascontiguousarray(context).astype(ml_dtypes.bfloat16)
    query = np.ascontiguousarray(query).astype(ml_dtypes.bfloat16)
    w = np.ascontiguousarray(w, dtype=np.float32)
    in_maps = []
    for i in range(NCORES):
        sl = slice(i * BPC, (i + 1) * BPC)
        in_maps.append({
            "context": context[sl],
            "query": query[sl],
            "w": w,
        })
    res = run_bass_kernel_spmd(_NC, in_maps, core_ids=list(range(NCORES)))
    out = np.concatenate(
        [np.asarray(r["out"]).astype(np.float32) for r in res.results],
        axis=0)
    out[:, 3 * D:4 * D, :] *= (1.0 / ST)
    return out


if __name__ == "__main__":
    rng = np.random.default_rng(0)
    out = kernel(
        context=rng.standard_normal((B, D, C), dtype=np.float32),
        query=rng.standard_normal((B, D, Q), dtype=np.float32),
        w=(rng.random(3 * D, dtype=np.float32) - 0.5) * 2 / np.sqrt(D),
    )
    print(out.shape, out.dtype)
